# revision 69
# baseline (speedup 1.0000x reference)
"""Multi-head self-attention with SDPA softcap, sharded over 8 NeuronCores.

Sharding: tensor-parallel over heads. Each core owns 2 of the 16 heads:
computes its q/k/v projections, attention (softcap tanh + softmax) for both
batches, and its row-slice of the output projection, producing a partial
[T, D] output summed on the host.

Datapath is fp16 end to end (11-bit mantissa ~ fp32r precision at half the
bytes; PE streams fp16 at 1 col/cycle like fp32r). exp uses a fixed -10
shift folded into the activation bias so unnormalized softmax weights fit
fp16 range; the shift cancels exactly in av/z.

v2 vs v1: single x read, q/k/v SBUF-resident (no DRAM roundtrip), 512-wide
streams everywhere, software-pipelined units so tensor/scalar/vector/DMA
overlap, fp16 halves DMA+SBUF traffic.
"""

import sys

if "/opt/trn_rl_repo" not in sys.path:
    sys.path.insert(0, "/opt/trn_rl_repo")

import numpy as np

import concourse.bass as bass
import concourse.bacc as bacc
import concourse.tile as tile
from concourse import mybir
from concourse.bass_utils import run_bass_kernel_spmd

F32 = mybir.dt.float32
F32R = mybir.dt.float32r
F16 = mybir.dt.float16

D = 2048          # model dim
H = 16            # total heads
DK = 128          # head dim
B = 2
S = 2048
T = B * S         # 4096 tokens
NCORES = 8
HC = 2            # heads per core
DPC = HC * DK     # 256

KC = D // 128     # 16 contraction chunks over model dim
KH = KC // 2      # 8: x column loaded in two halves
TCOL = 512        # projection token-column width
TQ = 512          # attention query-column width
NTQ = S // TQ     # 4 query cols per batch
NTK = S // 128    # 16 key blocks per batch
NB = S // 128     # 16 token blocks per batch
EXP_SHIFT = -10.0  # folded into exp bias; cancels in av/z


def _build_program(cap: float):
    nc = bacc.Bacc("TRN2", target_bir_lowering=False, debug=False,
                   num_devices=NCORES)

    # All inputs are pre-packed on the host so every DMA is fully
    # contiguous per partition (DMA here is packet-rate-bound: 1KB
    # descriptors cap at ~80/us, so layouts matter more than bytes).
    xP = nc.dram_tensor("xP", [128, T // TCOL, KC, TCOL], F16,
                        kind="ExternalInput").ap()
    ones_d = nc.dram_tensor("ones", [128, 128], F16, kind="ExternalInput").ap()
    wqP = nc.dram_tensor("wqP", [128, KC, DPC], F16, kind="ExternalInput").ap()
    wkP = nc.dram_tensor("wkP", [128, KC, DPC], F16, kind="ExternalInput").ap()
    wvP = nc.dram_tensor("wvP", [128, KC, DPC], F16, kind="ExternalInput").ap()
    woP = nc.dram_tensor("woP", [128, HC, 4, 512], F16,
                         kind="ExternalInput").ap()
    biasP = nc.dram_tensor("biasP", [128, NTQ, NTK, TQ], F16,
                           kind="ExternalInput").ap()
    out_d = nc.dram_tensor("out_partial", [T, D], F32R,
                           kind="ExternalOutput").ap()

    units = [(b, tqc) for b in range(B) for tqc in range(NTQ)]  # 8 units

    with tile.TileContext(nc) as tc:
        with (
            tc.tile_pool(name="const", bufs=1) as cpool,
            tc.tile_pool(name="pw", bufs=1) as pw,
            tc.tile_pool(name="pkv", bufs=1) as pkv,
            tc.tile_pool(name="px", bufs=2) as px,
            tc.tile_pool(name="pbias", bufs=2) as pbias,
            tc.tile_pool(name="ps", bufs=3) as ps_pool,
            tc.tile_pool(name="pot", bufs=1) as pot,
            tc.tile_pool(name="prz", bufs=2) as prz,
            tc.tile_pool(name="post", bufs=2) as post,
            tc.tile_pool(name="acc", bufs=2, space="PSUM") as acc,
        ):
            # Phase-scoped PSUM pools: projections (pa) live through E3,
            # then their banks are recycled for the wide P3 pool (po) whose
            # 2-bank tiles unlock the 2x-mode [128,1024] evacuation copies.
            pools = {}
            pools["sc"] = tc.alloc_tile_pool(name="sc", bufs=2, space="PSUM")
            pools["pa"] = tc.alloc_tile_pool(name="pa", bufs=2, space="PSUM")
            ones_sb = cpool.tile([128, 128], F16)
            shift_sb = cpool.tile([128, 1], F32)
            nc.gpsimd.memset(shift_sb[:], EXP_SHIFT)
            warm_sb = cpool.tile([128, 512], F16)
            nc.gpsimd.memset(warm_sb[:], 1.0)

            # Load order matters: the first k-matmul only needs wk + the
            # first x column, so issue those first and stagger the rest.
            wq_sb = pw.tile([128, KC, DPC], F16, tag="wq")
            wk_sb = pw.tile([128, KC, DPC], F16, tag="wk")
            wv_sb = pw.tile([128, KC, DPC], F16, tag="wv")
            wo_sb = pw.tile([128, HC, 4, 512], F16, tag="wo")
            nc.scalar.dma_start(out=wk_sb[:], in_=wkP[:])
            nc.sync.dma_start(out=ones_sb[:], in_=ones_d[:])

            def load_late_weights():
                nc.scalar.dma_start(out=wq_sb[:], in_=wqP[:])
                nc.scalar.dma_start(out=wv_sb[:], in_=wvP[:])

            def load_wo():
                nc.scalar.dma_start(out=wo_sb[:], in_=woP[:])

            qT_sb = pkv.tile([128, HC, T], F16, tag="qT")
            kT_sb = pkv.tile([128, HC, T], F16, tag="kT")
            vN_sb = pkv.tile([128, T // 128, DPC], F16, tag="vN")

            bias_map = {}
            s_map = {}
            ot_map = {}

            # ---------------- emission helpers ----------------

            def p1_tcol_start(tcol, after_dma=None, first=False):
                """Issue the x-column DMAs (two queues) and return the tile.
                The first column goes in quarters so the k-matmuls can
                start consuming as data arrives."""
                xt = px.tile([128, KC, TCOL], F16, tag="xc", name="xt")
                if first:
                    # eighths cycled over the three DMA-capable queues —
                    # per-queue bandwidth is the startup bottleneck and
                    # finer chunks keep arrival ahead of the k-matmuls
                    engs = (nc.sync, nc.gpsimd, nc.scalar)
                    for qi in range(8):
                        engs[qi % 3].dma_start(
                            out=xt[:, qi * 2:(qi + 1) * 2, :],
                            in_=xP[:, tcol, qi * 2:(qi + 1) * 2, :])
                else:
                    nc.sync.dma_start(
                        out=xt[:, 0:KH, :], in_=xP[:, tcol, 0:KH, :])
                    nc.gpsimd.dma_start(
                        out=xt[:, KH:KC, :], in_=xP[:, tcol, KH:KC, :])
                if after_dma is not None:
                    after_dma()
                return xt

            def p1_tcol_gen(tcol, xt):
                """Emit one 512-token projection column as a stream of
                4-matmul groups (32 yields) so it can interleave with the
                attention loops and fill tensor-queue bubbles."""
                t0 = tcol * TCOL
                for wsb, dest in ((wk_sb, kT_sb), (wq_sb, qT_sb)):
                    for m in range(HC):
                        psn = pools["pa"].tile([128, 512], F32, tag="pa",
                                               name="psn")
                        for kc in range(KC):
                            nc.tensor.matmul(
                                psn[:],
                                wsb[:, kc, m * 128:(m + 1) * 128],
                                xt[:, kc, :],
                                start=(kc == 0),
                                stop=(kc == KC - 1),
                            )
                            if kc % 4 == 3 and kc < KC - 1:
                                yield
                        nc.vector.tensor_copy(
                            dest[:, m, t0:t0 + TCOL], psn[:])
                        yield
                for g in range(2):  # token sub-block pairs
                    vp = pools["pa"].tile([128, 2, 256], F32, tag="pa",
                                          name="vp")
                    cnt = 0
                    for j in range(2):
                        tsub = g * 2 + j
                        for kc in range(KC):
                            nc.tensor.matmul(
                                vp[:, j, :],
                                xt[:, kc, tsub * 128:(tsub + 1) * 128],
                                wv_sb[:, kc, :],
                                start=(kc == 0),
                                stop=(kc == KC - 1),
                            )
                            cnt += 1
                            if cnt % 4 == 0 and cnt < 32:
                                yield
                    blk = t0 // 128 + g * 2
                    nc.vector.tensor_copy(vN_sb[:, blk:blk + 2, :], vp[:])
                    yield

            def p1_tcol(tcol, after_dma=None, first=False):
                xt = p1_tcol_start(tcol, after_dma, first=first)
                for _ in p1_tcol_gen(tcol, xt):
                    pass

            def bias_prefetch(u):
                if u >= len(units) or u in bias_map:
                    return
                b, tqc = units[u]
                halves = []
                for hb in range(2):
                    bc = pbias.tile([128, NTK // 2, TQ], F16, tag="bias",
                                    name="bc")
                    nc.scalar.dma_start(
                        out=bc[:],
                        in_=biasP[:, tqc,
                                  hb * (NTK // 2):(hb + 1) * (NTK // 2), :])
                    halves.append(bc)
                bias_map[u] = halves

            def emit_acts(u, h):
                """tanh + exp (in place) over one head's score tile."""
                s_t = s_map[(u, h)]
                flat = s_t[:].rearrange("p a b -> p (a b)")
                nc.scalar.activation(
                    flat, flat, mybir.ActivationFunctionType.Tanh,
                    scale=1.0 / cap)
                nc.scalar.activation(
                    flat, flat, mybir.ActivationFunctionType.Exp,
                    scale=cap, bias=shift_sb[:])

            sc_cur = {}

            def score_block(u, h, tkb):
                """Score blocks land in 2-bank PSUM pairs; one [128,1024]
                bias-add evacuates each pair (halves DVE instruction count)."""
                b, tqc = units[u]
                q0 = tqc * TQ
                if (u, h) not in s_map:
                    s_t = ps_pool.tile([128, NTK, TQ], F16, tag="s", name="s_t")
                    s_map[(u, h)] = s_t
                s_t = s_map[(u, h)]
                qcol = qT_sb[:, h, b * S + q0:b * S + q0 + TQ]
                if tkb % 2 == 0:
                    sc_cur[(u, h)] = pools["sc"].tile(
                        [128, 2, 512], F32, tag="sc", name="sps")
                sps = sc_cur[(u, h)]
                nc.tensor.matmul(
                    sps[:, tkb % 2, :],
                    kT_sb[:, h, b * S + tkb * 128:b * S + (tkb + 1) * 128],
                    qcol, start=True, stop=True)
                if tkb % 2 == 1:
                    hb = tkb // (NTK // 2)
                    bc = bias_map[u][hb]
                    ofs = (tkb - 1) % (NTK // 2)
                    nc.vector.tensor_add(
                        s_t[:, tkb - 1:tkb + 1, :], sps[:],
                        bc[:, ofs:ofs + 2, :])

            avz_state = {}

            def avz_block(u, h, tkb):
                b, tqc = units[u]
                if (u, h) not in avz_state:
                    avp = acc.tile([128, TQ], F32, tag="acc", name="avp")
                    zp = acc.tile([128, TQ], F32, tag="acc", name="zp")
                    avz_state[(u, h)] = (avp, zp)
                avp, zp = avz_state[(u, h)]
                s_t = s_map[(u, h)]
                nc.tensor.matmul(
                    avp[:],
                    vN_sb[:, b * NB + tkb, h * DK:(h + 1) * DK],
                    s_t[:, tkb, :],
                    start=(tkb == 0), stop=(tkb == NTK - 1))
                nc.tensor.matmul(
                    zp[:], ones_sb[:], s_t[:, tkb, :],
                    start=(tkb == 0), stop=(tkb == NTK - 1))

            def finish_head(u, h):
                avp, zp = avz_state.pop((u, h))
                s_map.pop((u, h))
                rz = prz.tile([128, TQ], F32, tag="rz", name="rz")
                nc.vector.reciprocal_approx_fast(out=rz[:], in_=zp[:])
                ot = pot.tile([128, TQ], F16, tag="ot", bufs=8, name="ot")
                nc.vector.tensor_mul(ot[:], avp[:], rz[:])
                ot_map[(u, h)] = ot

            def release_bias(u):
                bias_map.pop(u, None)

            def stage_out(u, drain=False):
                """Output projection + partial write for one unit.
                Two-bank PSUM tiles allow [128, 1024] 2x-mode evacuation
                copies; one DMA per 128-token block. In drain mode (kernel
                tail) evacuations alternate scalar/vector and DMAs go per
                pair so the tail isn't one serial chain."""
                if u < 0:
                    return
                b, tqc = units[u]
                o0 = ot_map.pop((u, 0))
                o1 = ot_map.pop((u, 1))
                for tb in range(TQ // 128):
                    gt = b * S + tqc * TQ + tb * 128
                    ost = post.tile([128, 4, 512], F32R, tag="ost", name="ost")
                    for ncc in range(4):
                        ps3 = pools["po"].tile([128, 512], F32, tag="po",
                                               name="ps3")
                        for hc, o in ((0, o0), (1, o1)):
                            nc.tensor.matmul(
                                ps3[:],
                                o[:, tb * 128:(tb + 1) * 128],
                                wo_sb[:, hc, ncc, :],
                                start=(hc == 0), stop=(hc == HC - 1))
                        if drain and ncc % 2 == 1:
                            nc.scalar.copy(ost[:, ncc, :], ps3[:])
                        else:
                            nc.vector.tensor_copy(ost[:, ncc, :], ps3[:])
                        if drain and ncc == 1:
                            nc.sync.dma_start(
                                out=out_d[gt:gt + 128, 0:1024],
                                in_=ost[:, 0:2, :])
                    if drain:
                        nc.gpsimd.dma_start(
                            out=out_d[gt:gt + 128, 1024:2048],
                            in_=ost[:, 2:4, :])
                    else:
                        nc.gpsimd.dma_start(
                            out=out_d[gt:gt + 128, :], in_=ost[:])

            def unit_loops(u, prev, filler=None):
                """Three-phase step: (A) scores h0, (B) av/z of prev h0,
                (C) scores h1 + av/z prev h1. A dense-matmul filler stream
                (batch-1 projections) interleaves into A and B so the
                tensor queue never stalls behind the DVE bias-adds, and
                av/z never head-of-line-blocks it waiting on exp."""
                box = [filler]

                def take():
                    if box[0] is not None:
                        try:
                            next(box[0])
                        except StopIteration:
                            box[0] = None

                if prev >= 0:
                    for h in range(HC):
                        emit_acts(prev, h)
                for tkb in range(NTK):          # A
                    if u < len(units):
                        score_block(u, 0, tkb)
                    take()
                for tkb in range(NTK):          # B
                    if prev >= 0:
                        avz_block(prev, 0, tkb)
                    take()
                if prev >= 0:
                    finish_head(prev, 0)
                for tkb in range(NTK):          # C
                    if u < len(units):
                        score_block(u, 1, tkb)
                    if prev >= 0:
                        avz_block(prev, 1, tkb)
                    take()
                if prev >= 0:
                    finish_head(prev, 1)
                    release_bias(prev)
                while box[0] is not None:
                    take()

            # ---------------- program ----------------

            # Warm the PE clock (HAM un-throttles after ~3.4us of activity)
            # while the first weight/x DMAs are in flight; outputs unused.
            # Sized to end just as the first x column lands so the real
            # matmuls start at 2.4 GHz with no re-throttling idle window.
            for wi in range(40):
                wps = pools["pa"].tile([128, 512], F32, tag="pa", name="wps")
                nc.tensor.matmul(
                    wps[:], warm_sb[:, 0:128], warm_sb[:],
                    start=True, stop=True)

            # batch-0 projections (dense tensor work)
            p1_tcol(0, after_dma=load_late_weights, first=True)
            p1_tcol(1, after_dma=load_wo)
            bias_prefetch(0)
            bias_prefetch(1)
            for tcol in range(2, 4):
                p1_tcol(tcol)

            # pipelined middle: batch-1 projections stream into the
            # batch-0 attention loops as tensor-queue filler
            for step in range(4):
                u = step               # current unit entering scores
                xt = p1_tcol_start(4 + step)
                unit_loops(u, u - 1, filler=p1_tcol_gen(4 + step, xt))
                bias_prefetch(u + 2)

            # recycle projection PSUM banks into the wide P3 pool
            pools["pa"].release()
            pools["po"] = tc.alloc_tile_pool(name="po", bufs=2, space="PSUM")

            # batch-1 attention + deferred batch-0 output projections
            out_sched = {4: (), 5: (0, 1), 6: (2, 3), 7: (4, 5)}
            for step in range(4, 8):
                u = step
                for ou in out_sched[step]:
                    stage_out(ou)
                unit_loops(u, u - 1)
                bias_prefetch(u + 2)

            # epilogue: drain last unit — activations first so the scalar
            # engine runs under out(6)'s tensor work
            for h in range(HC):
                emit_acts(7, h)
            stage_out(6)
            for h in range(HC):
                for tkb in range(NTK):
                    avz_block(7, h, tkb)
                finish_head(7, h)
            release_bias(7)
            stage_out(7, drain=True)
            pools["po"].release()
            pools["sc"].release()

    nc.compile()
    return nc


_PROGRAM_CACHE: dict = {}


def _get_program(cap: float):
    if cap not in _PROGRAM_CACHE:
        _PROGRAM_CACHE[cap] = _build_program(cap)
    return _PROGRAM_CACHE[cap]


def _pack_contract(mat):
    """[D, N] -> [128, KC, N] partition-major over the contraction dim."""
    return np.ascontiguousarray(
        mat.reshape(KC, 128, -1).transpose(1, 0, 2)).astype(np.float16)


def _prepare_in_maps(inp, wq, wk, wv, wo, attn_bias, softcap):
    x = np.ascontiguousarray(np.asarray(inp, dtype=np.float32)).reshape(T, D)
    # x: [128, KC, T] -> [128, n_tcol, KC, TCOL] (contiguous per column)
    xP = np.ascontiguousarray(
        _pack_contract(np.ascontiguousarray(x.T))
        .reshape(128, KC, T // TCOL, TCOL).transpose(0, 2, 1, 3))
    # bias: [128, NTK, S] -> [128, NTQ, NTK, TQ]
    biasP = np.ascontiguousarray(
        _pack_contract(
            np.ascontiguousarray(
                np.asarray(attn_bias, dtype=np.float32).reshape(S, S).T))
        .reshape(128, NTK, NTQ, TQ).transpose(0, 2, 1, 3))
    wq = np.asarray(wq, dtype=np.float32)
    wk = np.asarray(wk, dtype=np.float32)
    wv = np.asarray(wv, dtype=np.float32)
    wo = np.asarray(wo, dtype=np.float32)
    scale = 1.0 / np.sqrt(np.float32(DK))

    in_maps = []
    for c in range(NCORES):
        rows = slice(c * DPC, (c + 1) * DPC)
        # woP: [128 (dk within head), HC, 4, 512]
        woc = wo[:, rows].T.reshape(HC, 128, 4, 512).transpose(1, 0, 2, 3)
        in_maps.append({
            "xP": xP,
            "ones": np.ones((128, 128), dtype=np.float16),
            "wqP": _pack_contract(np.ascontiguousarray((wq[rows] * scale).T)),
            "wkP": _pack_contract(np.ascontiguousarray(wk[rows].T)),
            "wvP": _pack_contract(np.ascontiguousarray(wv[rows].T)),
            "woP": np.ascontiguousarray(woc).astype(np.float16),
            "biasP": biasP,
        })
    return in_maps


def run(inputs: dict, trace: bool = False):
    """Run the SPMD kernel. Returns (full_output, BassKernelResults)."""
    cap = float(inputs["softcap"])
    nc = _get_program(cap)
    in_maps = _prepare_in_maps(
        inputs["inp"], inputs["wq"], inputs["wk"], inputs["wv"],
        inputs["wo"], inputs["attn_bias"], inputs["softcap"],
    )
    res = run_bass_kernel_spmd(
        nc, in_maps, list(range(NCORES)), trace=trace,
    )
    acc = np.zeros((T, D), dtype=np.float64)
    for c in range(NCORES):
        acc += res.results[c]["out_partial"]
    out = acc.astype(np.float32).reshape(B, S, D)
    return out, res


def kernel(**inputs) -> np.ndarray:
    out, _ = run(inputs, trace=False)
    return out


if __name__ == "__main__":
    rng = np.random.default_rng(0)
    sc = 1.0 / np.sqrt(D)
    inputs = {
        "inp": rng.standard_normal((B, S, D)).astype(np.float32),
        "wq": (rng.standard_normal((D, D)) * sc).astype(np.float32),
        "wk": (rng.standard_normal((D, D)) * sc).astype(np.float32),
        "wv": (rng.standard_normal((D, D)) * sc).astype(np.float32),
        "wo": (rng.standard_normal((D, D)) * sc).astype(np.float32),
        "attn_bias": rng.standard_normal((1, 1, S, S)).astype(np.float32),
        "softcap": 30,
    }
    out = kernel(**inputs)
    print("out", out.shape, out.dtype, float(np.abs(out).max()))
